# revision 6
# baseline (speedup 1.0000x reference)
"""Trainium2 Bass kernel for nn_Attention_944892805701.

Dense transformer attention layer: QKV projection + RoPE + causal GQA SDPA +
output projection. B=2, S=2048, DIM=4096, 32 Q heads / 8 KV heads, hd=128.

Sharding (8 cores): 2 (batch) x 4 (head groups). Core (b, g) computes global
Q heads [8g, 8g+8) / KV heads [2g, 2g+2) of batch b and the corresponding
partial output projection y_partial = att_heads @ Wo[:, o_slice]^T. The host
sums the 4 head-group partials per batch (the tensor-parallel "allreduce",
done on host since full outputs are gathered there anyway).

Per-core device program (all f32 storage; matmuls run as float32r = full
PE rate for moving dim >= 256):
  Phase 1 (per 512-wide s-chunk): PE-transpose x into xT [din, s] tiles,
    project qT/kT in [head_dim, s] layout (RoPE fused into the PSUM drain)
    and vT, which is PE-transposed again into natural [s, d] tiles.
  Phase 2 (per q-chunk, per head): scoresT in [k-part, q-free] layout
    (lhsT = kT tile, rhs = qT chunk), exp on ScalarE with the 1/sqrt(hd)
    scale folded in, causality via restricted column ranges plus one
    triangular 128x128 mask multiply per diagonal block, softmax
    denominator accumulated on DVE and partition-reduced with a ones
    matmul, reciprocal broadcast via gpsimd partition_broadcast, and the
    normalization fused into the PV PSUM drain. Normalized attention
    outputs (attT, [local_heads*128, S]) round-trip through DRAM.
  Phase 3: outT[m,:] = sum_o WoT[o-tile, m-tile].T @ attT[o-tile, :].

Output per core: outT [4096, 2048] = y_partial^T; host transposes + sums.
"""

import math
from contextlib import ExitStack

import numpy as np

import concourse.bass as bass  # noqa: F401  (bass types via bacc/tile)
import concourse.tile as tile
from concourse import bacc, mybir
from concourse.bass_utils import run_bass_kernel_spmd

F32 = mybir.dt.float32
F32R = mybir.dt.float32r

N_CORES = 8
DIM = 4096
N_HEADS = 32
N_KV_HEADS = 8
HEAD_DIM = 128
SEQ = 2048

HQ = N_HEADS // 4      # 8 local q heads (4 head groups)
HKV = N_KV_HEADS // 4  # 2 local kv heads

SC = 512               # s-chunk width (matmul moving size)
P = 128


def _r(ap):
    return ap.bitcast(F32R)


def build_program(seq=SEQ, dim=DIM, hq=HQ, hkv=HKV):
    """Build the per-core Bass program. seq % 512 == 0, dim % 1024 == 0."""
    nrep = hq // hkv
    nch = seq // SC          # q/s chunks
    ndt = dim // P           # din tiles
    nkt_total = seq // P     # k tiles
    dq = hq * HEAD_DIM       # local q width (1024)
    dkv = hkv * HEAD_DIM     # local kv width (256)
    scale = 1.0 / math.sqrt(HEAD_DIM)

    nc = bacc.Bacc("TRN2", target_bir_lowering=False, debug=False,
                   num_devices=N_CORES)

    x = nc.dram_tensor("x", [seq, dim], F32, kind="ExternalInput").ap()
    wqT = nc.dram_tensor("wqT", [dim, dq], F32R, kind="ExternalInput").ap()
    wkT = nc.dram_tensor("wkT", [dim, dkv], F32R, kind="ExternalInput").ap()
    wvT = nc.dram_tensor("wvT", [dim, dkv], F32R, kind="ExternalInput").ap()
    # wot[m] = WoT_shard[:, 128m:128m+128]  -> [dim/128, dq, 128]
    wot = nc.dram_tensor("wot", [dim // P, dq, P], F32R,
                         kind="ExternalInput").ap()
    cosT = nc.dram_tensor("cosT", [HEAD_DIM, seq], F32,
                          kind="ExternalInput").ap()
    sinT = nc.dram_tensor("sinT", [HEAD_DIM, seq], F32,
                          kind="ExternalInput").ap()
    # tri[p, j] = 1.0 if j >= p else 0.0  (keep k <= q in scoresT layout)
    tri = nc.dram_tensor("tri", [P, P], F32, kind="ExternalInput").ap()
    iden = nc.dram_tensor("iden", [P, P], F32, kind="ExternalInput").ap()
    ones_col = nc.dram_tensor("ones_col", [P, 1], F32R,
                              kind="ExternalInput").ap()
    outT = nc.dram_tensor("outT", [dim, seq], F32, kind="ExternalOutput").ap()

    with ExitStack() as ctx:
        tc = ctx.enter_context(tile.TileContext(nc))
        # [128, 512] f32 workspace: xT / exp / acc / drains / att_sb
        ws = ctx.enter_context(tc.tile_pool(name="ws", bufs=39))
        # [128, seq] persistent qTr/kTr
        big = ctx.enter_context(tc.tile_pool(name="big", bufs=hq + hkv))
        vp = ctx.enter_context(tc.tile_pool(name="vp", bufs=hkv * nkt_total))
        xr = ctx.enter_context(tc.tile_pool(name="xr", bufs=3))
        wqp = ctx.enter_context(tc.tile_pool(name="wqp", bufs=3))
        wkvp = ctx.enter_context(tc.tile_pool(name="wkvp", bufs=4))
        wop = ctx.enter_context(tc.tile_pool(name="wop", bufs=2))
        cns = ctx.enter_context(tc.tile_pool(name="cns", bufs=1))
        # PSUM: "s"(2) + "o"(2) | "p"(2) + "t"(2)  = 8 banks
        ps_a = ctx.enter_context(tc.tile_pool(name="ps_a", bufs=2,
                                              space="PSUM"))
        ps_p = ctx.enter_context(tc.tile_pool(name="ps_p", bufs=2,
                                              space="PSUM"))
        dram = ctx.enter_context(tc.tile_pool(name="dram", bufs=1,
                                              space="DRAM"))

        attT = dram.tile([dq, seq], F32R)

        tri_sb = cns.tile([P, P], F32, tag="tri")
        nc.sync.dma_start(tri_sb[:], tri[:])
        iden_sb = cns.tile([P, P], F32, tag="iden")
        nc.sync.dma_start(iden_sb[:], iden[:])
        ones_sb = cns.tile([P, 1], F32R, tag="ones")
        nc.sync.dma_start(ones_sb[:], ones_col[:])

        qTr = [big.tile([P, seq], F32, tag="big", name=f"qTr{i}") for i in range(hq)]
        kTr = [big.tile([P, seq], F32, tag="big", name=f"kTr{i}") for i in range(hkv)]
        v_nat = [[vp.tile([P, HEAD_DIM], F32, tag="v", name=f"v{g}_{t}")
                  for t in range(nkt_total)] for g in range(hkv)]

        def rope_drain(dst, psum, cos_c, sin_c):
            """dst = psum*cos + rotate_half(psum)*sin  ([128, SC] drain)."""
            h = HEAD_DIM // 2
            tmp = ws.tile([P, SC], F32, tag="ws")
            nc.vector.tensor_mul(_r(dst), psum, cos_c[:])
            nc.vector.tensor_mul(tmp[0:h, :], psum[h:P, :], sin_c[0:h, :])
            nc.vector.tensor_mul(tmp[h:P, :], psum[0:h, :], sin_c[h:P, :])
            nc.vector.tensor_sub(_r(dst[0:h, :]), dst[0:h, :], tmp[0:h, :])
            nc.vector.tensor_add(_r(dst[h:P, :]), dst[h:P, :], tmp[h:P, :])

        for c in range(nch):
            s0 = c * SC

            cos_c = ws.tile([P, SC], F32, tag="ws")
            nc.sync.dma_start(cos_c[:], cosT[:, s0:s0 + SC])
            sin_c = ws.tile([P, SC], F32, tag="ws")
            nc.sync.dma_start(sin_c[:], sinT[:, s0:s0 + SC])

            # --- transpose x chunk into xT [din-tile][128, SC]
            xT = [ws.tile([P, SC], F32, tag="ws", name=f"xT{c}_{i}") for i in range(ndt)]
            for st in range(SC // P):
                for j in range(dim // 1024):
                    row = xr.tile([P, 1024], F32, tag="xr")
                    nc.sync.dma_start(
                        row[:], x[s0 + st * P:s0 + (st + 1) * P,
                                  j * 1024:(j + 1) * 1024])
                    for t in range(1024 // P):
                        dt = j * (1024 // P) + t
                        pt = ps_p.tile([P, P], F32, tag="t")
                        nc.tensor.transpose(pt[:], row[:, t * P:(t + 1) * P],
                                            iden_sb[:])
                        nc.any.tensor_copy(_r(xT[dt][:, st * P:(st + 1) * P]),
                                           pt[:])

            # --- q projection (head pairs share one W block load)
            for hb in range(0, hq, 2):
                pqs = [ps_p.tile([P, SC], F32, tag="p", name=f"pq{c}_{hb}_{i}") for i in range(2)]
                for dt in range(ndt):
                    wq = wqp.tile([P, 2 * HEAD_DIM], F32R, tag="wq")
                    nc.sync.dma_start(
                        wq[:], wqT[dt * P:(dt + 1) * P,
                                   hb * HEAD_DIM:(hb + 2) * HEAD_DIM])
                    for i in range(2):
                        nc.tensor.matmul(
                            pqs[i][:],
                            wq[:, i * HEAD_DIM:(i + 1) * HEAD_DIM],
                            _r(xT[dt][:]),
                            start=(dt == 0), stop=(dt == ndt - 1))
                for i in range(2):
                    rope_drain(qTr[hb + i][:, s0:s0 + SC], pqs[i][:],
                               cos_c, sin_c)

            # --- k projection
            pks = [ps_p.tile([P, SC], F32, tag="p", name=f"pk{c}_{i}") for i in range(hkv)]
            for dt in range(ndt):
                wk = wkvp.tile([P, dkv], F32R, tag="wkv")
                nc.sync.dma_start(wk[:], wkT[dt * P:(dt + 1) * P, :])
                for g in range(hkv):
                    nc.tensor.matmul(
                        pks[g][:],
                        wk[:, g * HEAD_DIM:(g + 1) * HEAD_DIM],
                        _r(xT[dt][:]),
                        start=(dt == 0), stop=(dt == ndt - 1))
            for g in range(hkv):
                rope_drain(kTr[g][:, s0:s0 + SC], pks[g][:], cos_c, sin_c)

            # --- v projection (transposed), then PE-transpose to [s, d]
            pvs = [ps_p.tile([P, SC], F32, tag="p", name=f"pv{c}_{i}") for i in range(hkv)]
            for dt in range(ndt):
                wv = wkvp.tile([P, dkv], F32R, tag="wkv")
                nc.sync.dma_start(wv[:], wvT[dt * P:(dt + 1) * P, :])
                for g in range(hkv):
                    nc.tensor.matmul(
                        pvs[g][:],
                        wv[:, g * HEAD_DIM:(g + 1) * HEAD_DIM],
                        _r(xT[dt][:]),
                        start=(dt == 0), stop=(dt == ndt - 1))
            for g in range(hkv):
                vt_sb = ws.tile([P, SC], F32, tag="ws")
                nc.any.tensor_copy(vt_sb[:], pvs[g][:])
                for st in range(SC // P):
                    pt = ps_p.tile([P, P], F32, tag="t")
                    nc.tensor.transpose(pt[:], vt_sb[:, st * P:(st + 1) * P],
                                        iden_sb[:])
                    nc.any.tensor_copy(_r(v_nat[g][c * (SC // P) + st][:]), pt[:])

            # --- Phase 2: attention for this q-chunk, all local heads
            nkt = (c + 1) * (SC // P)
            for h in range(hq):
                g = h // nrep
                acc = ws.tile([P, SC], F32, tag="ws")
                po = ps_a.tile([P, SC], F32, tag="o")
                for kt in range(nkt):
                    rr = kt * P - s0          # k0 - q0 offset
                    jlo = max(0, rr)
                    pscr = ps_a.tile([P, SC], F32, tag="s")
                    nc.tensor.matmul(
                        pscr[:, jlo:SC],
                        _r(kTr[g][:, kt * P:(kt + 1) * P]),
                        _r(qTr[h][:, s0 + jlo:s0 + SC]),
                        start=True, stop=True)
                    et = ws.tile([P, SC], F32, tag="ws")
                    nc.scalar.activation(
                        _r(et[:, jlo:SC]), pscr[:, jlo:SC],
                        mybir.ActivationFunctionType.Exp, scale=scale)
                    if rr >= 0:
                        nc.vector.tensor_mul(_r(et[:, jlo:jlo + P]),
                                             et[:, jlo:jlo + P], tri_sb[:])
                    if kt == 0:
                        nc.vector.tensor_copy(_r(acc[:]), et[:])
                    else:
                        nc.vector.tensor_add(_r(acc[:, jlo:SC]), acc[:, jlo:SC],
                                             et[:, jlo:SC])
                    nc.tensor.matmul(
                        po[:, jlo:SC],
                        _r(v_nat[g][kt][:]),
                        _r(et[:, jlo:SC]),
                        start=(kt == 0), stop=(kt == nkt - 1))
                # denominator: cross-partition sum via ones-matmul
                pd = ps_p.tile([P, SC], F32, tag="t")
                nc.tensor.matmul(pd[0:1, :], ones_sb[:], _r(acc[:]),
                                 start=True, stop=True)
                rd = ws.tile([P, SC], F32, tag="ws")
                nc.vector.reciprocal(rd[0:1, :], pd[0:1, :])
                rb = ws.tile([P, SC], F32, tag="ws")
                nc.gpsimd.partition_broadcast(rb[:], rd[0:1, :])
                ao = ws.tile([P, SC], F32R, tag="ws")
                nc.vector.tensor_mul(ao[:], po[:], rb[:])
                nc.sync.dma_start(attT[h * P:(h + 1) * P, s0:s0 + SC], ao[:])

        # --- Phase 3: output projection
        att_sb = [[None] * nch for _ in range(hq)]
        for o in range(hq):
            for cc in range(nch):
                t = ws.tile([P, SC], F32R, tag="ws", name=f"att_sb{o}_{cc}")
                nc.sync.dma_start(
                    t[:], attT[o * P:(o + 1) * P, cc * SC:(cc + 1) * SC])
                att_sb[o][cc] = t
        for m in range(dim // P):
            wo = wop.tile([P, hq, P], F32R, tag="wo")
            nc.sync.dma_start(wo[:], wot[m].rearrange("(o p) f -> p o f", p=P))
            for cc in range(nch):
                py = ps_a.tile([P, SC], F32, tag="s")
                for o in range(hq):
                    nc.tensor.matmul(
                        py[:], wo[:, o, :], att_sb[o][cc][:],
                        start=(o == 0), stop=(o == hq - 1))
                yo = ws.tile([P, SC], F32, tag="ws")
                nc.any.tensor_copy(yo[:], py[:])
                nc.sync.dma_start(
                    outT[m * P:(m + 1) * P, cc * SC:(cc + 1) * SC], yo[:])

    nc.compile()
    return nc


def make_core_inputs(data, Wq, Wk, Wv, Wo, cos, sin):
    """Build in_maps for the 8 cores. Core id = 4*b + g."""
    c = np.ascontiguousarray
    dq = HQ * HEAD_DIM
    dkv = HKV * HEAD_DIM
    dim = Wq.shape[1]
    tri_m = np.triu(np.ones((P, P), dtype=np.float32))
    iden = np.eye(P, dtype=np.float32)
    ones_col = np.ones((P, 1), dtype=np.float32)
    cosT = c(cos.T.astype(np.float32))
    sinT = c(sin.T.astype(np.float32))
    in_maps = []
    for core in range(N_CORES):
        b, g = divmod(core, 4)
        qs = slice(g * dq, (g + 1) * dq)
        ks = slice(g * dkv, (g + 1) * dkv)
        woT = c(Wo[:, qs].T)                     # [dq, dim]
        wot = c(woT.reshape(dq, dim // P, P).transpose(1, 0, 2))
        in_maps.append({
            "x": c(data[b]),
            "wqT": c(Wq[qs, :].T),
            "wkT": c(Wk[ks, :].T),
            "wvT": c(Wv[ks, :].T),
            "wot": wot,
            "cosT": cosT,
            "sinT": sinT,
            "tri": tri_m,
            "iden": iden,
            "ones_col": ones_col,
        })
    return in_maps


_COMPILED = {}


def _get_program():
    key = (SEQ, DIM, HQ, HKV)
    if key not in _COMPILED:
        _COMPILED[key] = build_program()
    return _COMPILED[key]


def run(inputs, trace=False, tmpdir=None, trace_cores=None):
    nc = _get_program()
    in_maps = make_core_inputs(
        inputs["data"], inputs["Wq"], inputs["Wk"], inputs["Wv"],
        inputs["Wo"], inputs["cos"], inputs["sin"])
    kw = {}
    if trace:
        kw = dict(trace=True, tmpdir=tmpdir, trace_cores=trace_cores)
    res = run_bass_kernel_spmd(nc, in_maps, list(range(N_CORES)), **kw)
    B = inputs["data"].shape[0]
    out = np.zeros((B, SEQ, DIM), dtype=np.float32)
    for core in range(N_CORES):
        b = core // 4
        out[b] += res.results[core]["outT"].T
    return out, res


def kernel(data, Wq, Wk, Wv, Wo, cos, sin, mask):
    assert np.asarray(mask).size == 1, "only causal (numel==1) mask supported"
    inputs = {
        "data": np.asarray(data, dtype=np.float32),
        "Wq": np.asarray(Wq, dtype=np.float32),
        "Wk": np.asarray(Wk, dtype=np.float32),
        "Wv": np.asarray(Wv, dtype=np.float32),
        "Wo": np.asarray(Wo, dtype=np.float32),
        "cos": np.asarray(cos, dtype=np.float32),
        "sin": np.asarray(sin, dtype=np.float32),
    }
    out, _ = run(inputs)
    return out


# revision 9
# speedup vs baseline: 1.2119x; 1.2119x over previous
"""Trainium2 Bass kernel for nn_Attention_944892805701.

Dense transformer attention layer: QKV projection + RoPE + causal GQA SDPA +
output projection. B=2, S=2048, DIM=4096, 32 Q heads / 8 KV heads, hd=128.

Sharding (8 cores): 2 (batch) x 4 (head groups). Core (b, g) computes global
Q heads [8g, 8g+8) / KV heads [2g, 2g+2) of batch b and the corresponding
partial output projection y_partial = att_heads @ Wo[:, o_slice]^T. The host
sums the 4 head-group partials per batch (the tensor-parallel "allreduce",
done on host since full outputs are gathered there anyway).

Per-core device program: bf16 matmul operands (full PE rate + FWL weight
loads; fp32r measured ~2 cyc/row on HW so bf16 is 2x faster), fp32 PSUM
accumulation everywhere, fp32 softmax statistics.

  Phase 1 (per 512-wide s-chunk): xT tiles [din, s] via bf16 DMA transpose
    straight from DRAM (host pre-casts x to bf16), project qT/kT in
    [head_dim, s] layout (RoPE fused into the fp32 PSUM drain, bf16 out)
    and vT -> PE-transposed into natural [s, d] bf16 tiles.
  Phase 2 (per q-chunk, per head): scoresT = kT_tile x qT_chunk in
    [k-part, q-free] layout, exp on ScalarE (1/sqrt(hd) folded into the
    activation scale), causality via restricted column ranges plus one
    triangular mask multiply per diagonal block, denominator = fp32 DVE
    accumulation + fp32r ones-matmul partition reduce, PV accumulated in
    PSUM and drained UNNORMALIZED (bf16) to persistent SBUF tiles.
    Denominators go to DRAM ([1,512] rows); after each chunk one batched
    [8,512] reciprocal + partition_broadcast normalizes the chunk's ao
    tiles in place (off the per-head critical path).
  Phase 3: outT[m,:] = sum_o WoT[o-tile, m-tile].T @ att[o-tile, :] from
    SBUF-resident normalized bf16 attention tiles.

Output per core: outT [4096, 2048] f32 = y_partial^T; host transposes+sums.
"""

import math
from contextlib import ExitStack

import numpy as np
import ml_dtypes

import concourse.bass as bass  # noqa: F401
import concourse.tile as tile
from concourse import bacc, mybir
from concourse.bass_utils import run_bass_kernel_spmd

F32 = mybir.dt.float32
F32R = mybir.dt.float32r
BF16 = mybir.dt.bfloat16

N_CORES = 8
DIM = 4096
N_HEADS = 32
N_KV_HEADS = 8
HEAD_DIM = 128
SEQ = 2048

HQ = N_HEADS // 4      # 8 local q heads
HKV = N_KV_HEADS // 4  # 2 local kv heads

SC = 512
P = 128


def _r(ap):
    return ap.bitcast(F32R)


def build_program(seq=SEQ, dim=DIM, hq=HQ, hkv=HKV, debug=False):
    nrep = hq // hkv
    nch = seq // SC
    ndt = dim // P
    nkt_total = seq // P
    dq = hq * HEAD_DIM
    dkv = hkv * HEAD_DIM
    scale = 1.0 / math.sqrt(HEAD_DIM)

    nc = bacc.Bacc("TRN2", target_bir_lowering=False, debug=False,
                   num_devices=N_CORES)

    x = nc.dram_tensor("x", [seq, dim], BF16, kind="ExternalInput").ap()
    wqT = nc.dram_tensor("wqT", [dim, dq], BF16, kind="ExternalInput").ap()
    wkT = nc.dram_tensor("wkT", [dim, dkv], BF16, kind="ExternalInput").ap()
    wvT = nc.dram_tensor("wvT", [dim, dkv], BF16, kind="ExternalInput").ap()
    wot = nc.dram_tensor("wot", [dim // P, dq, P], BF16,
                         kind="ExternalInput").ap()
    cosT = nc.dram_tensor("cosT", [HEAD_DIM, seq], F32,
                          kind="ExternalInput").ap()
    sinT = nc.dram_tensor("sinT", [HEAD_DIM, seq], F32,
                          kind="ExternalInput").ap()
    tri = nc.dram_tensor("tri", [P, P], BF16, kind="ExternalInput").ap()
    iden = nc.dram_tensor("iden", [P, P], BF16, kind="ExternalInput").ap()
    ones_col = nc.dram_tensor("ones_col", [P, 1], F32R,
                              kind="ExternalInput").ap()
    outT = nc.dram_tensor("outT", [dim, seq], F32, kind="ExternalOutput").ap()
    dbg = {}
    if debug:
        for nm in ("dq0", "dk0"):
            dbg[nm] = nc.dram_tensor(nm, [P, seq], BF16,
                                     kind="ExternalOutput").ap()
        dbg["dv0"] = nc.dram_tensor("dv0", [P, HEAD_DIM], BF16,
                                    kind="ExternalOutput").ap()
        dbg["dao0"] = nc.dram_tensor("dao0", [P, SC], BF16,
                                     kind="ExternalOutput").ap()
        dbg["ddn"] = nc.dram_tensor("ddn", [hq, seq], F32,
                                    kind="ExternalOutput").ap()
        dbg["det0"] = nc.dram_tensor("det0", [P, SC], BF16,
                                     kind="ExternalOutput").ap()

    with ExitStack() as ctx:
        tc = ctx.enter_context(tile.TileContext(nc))
        ws = ctx.enter_context(tc.tile_pool(name="ws", bufs=16))    # f32 512
        wsb = ctx.enter_context(tc.tile_pool(name="wsb", bufs=76))  # bf16 512
        big = ctx.enter_context(tc.tile_pool(name="big", bufs=hq + hkv))
        vp = ctx.enter_context(tc.tile_pool(name="vp", bufs=hkv * nkt_total))
        wqp = ctx.enter_context(tc.tile_pool(name="wqp", bufs=4))
        wkvp = ctx.enter_context(tc.tile_pool(name="wkvp", bufs=6))
        wop = ctx.enter_context(tc.tile_pool(name="wop", bufs=3))
        cns = ctx.enter_context(tc.tile_pool(name="cns", bufs=1))
        ps_a = ctx.enter_context(tc.tile_pool(name="ps_a", bufs=2,
                                              space="PSUM"))
        ps_p = ctx.enter_context(tc.tile_pool(name="ps_p", bufs=2,
                                              space="PSUM"))
        dram = ctx.enter_context(tc.tile_pool(name="dram", bufs=1,
                                              space="DRAM"))

        dn_dram = dram.tile([hq, seq], F32, tag="dn")
        dnr_dram = dram.tile([hq, seq], F32, tag="dnr")

        tri_sb = cns.tile([P, P], BF16, tag="tri")
        nc.sync.dma_start(tri_sb[:], tri[:])
        iden_sb = cns.tile([P, P], BF16, tag="iden")
        nc.sync.dma_start(iden_sb[:], iden[:])
        ones_sb = cns.tile([P, 1], F32R, tag="ones")
        nc.sync.dma_start(ones_sb[:], ones_col[:])

        qTr = [big.tile([P, seq], BF16, tag="big", name=f"qTr{i}")
               for i in range(hq)]
        kTr = [big.tile([P, seq], BF16, tag="big", name=f"kTr{i}")
               for i in range(hkv)]
        v_nat = [[vp.tile([P, HEAD_DIM], BF16, tag="v", name=f"v{g}_{t}")
                  for t in range(nkt_total)] for g in range(hkv)]
        # unnormalized attention output tiles, persistent through phase 3
        ao = [[wsb.tile([P, SC], BF16, tag="wsb", name=f"ao{h}_{cc}")
               for cc in range(nch)] for h in range(hq)]

        def rope_drain(dst, psum, cos_c, sin_c):
            """dst(bf16) = psum*cos + rotate_half(psum)*sin."""
            h = HEAD_DIM // 2
            tmp = ws.tile([P, SC], F32, tag="ws")
            nc.vector.tensor_mul(dst, psum, cos_c[:])
            nc.vector.tensor_mul(tmp[0:h, :], psum[h:P, :], sin_c[0:h, :])
            nc.vector.tensor_mul(tmp[h:P, :], psum[0:h, :], sin_c[h:P, :])
            nc.vector.tensor_sub(dst[0:h, :], dst[0:h, :], tmp[0:h, :])
            nc.vector.tensor_add(dst[h:P, :], dst[h:P, :], tmp[h:P, :])

        for c in range(nch):
            s0 = c * SC

            cos_c = ws.tile([P, SC], F32, tag="ws")
            nc.sync.dma_start(cos_c[:], cosT[:, s0:s0 + SC])
            sin_c = ws.tile([P, SC], F32, tag="ws")
            nc.sync.dma_start(sin_c[:], sinT[:, s0:s0 + SC])

            # xT via bf16 DMA transpose straight from DRAM
            xT = [wsb.tile([P, SC], BF16, tag="wsb", name=f"xT{c}_{i}")
                  for i in range(ndt)]
            for dt in range(ndt):
                nc.sync.dma_start_transpose(
                    xT[dt][:], x[s0:s0 + SC, dt * P:(dt + 1) * P])

            # q projection: head pairs share one W block load
            for hb in range(0, hq, 2):
                pqs = [ps_p.tile([P, SC], F32, tag="p",
                                 name=f"pq{c}_{hb}_{i}") for i in range(2)]
                for dt in range(ndt):
                    wq = wqp.tile([P, 2 * HEAD_DIM], BF16, tag="wq")
                    nc.sync.dma_start(
                        wq[:], wqT[dt * P:(dt + 1) * P,
                                   hb * HEAD_DIM:(hb + 2) * HEAD_DIM])
                    for i in range(2):
                        nc.tensor.matmul(
                            pqs[i][:],
                            wq[:, i * HEAD_DIM:(i + 1) * HEAD_DIM],
                            xT[dt][:],
                            start=(dt == 0), stop=(dt == ndt - 1))
                for i in range(2):
                    rope_drain(qTr[hb + i][:, s0:s0 + SC], pqs[i][:],
                               cos_c, sin_c)

            # k projection
            pks = [ps_p.tile([P, SC], F32, tag="p", name=f"pk{c}_{i}")
                   for i in range(hkv)]
            for dt in range(ndt):
                wk = wkvp.tile([P, dkv], BF16, tag="wkv")
                nc.sync.dma_start(wk[:], wkT[dt * P:(dt + 1) * P, :])
                for g in range(hkv):
                    nc.tensor.matmul(
                        pks[g][:],
                        wk[:, g * HEAD_DIM:(g + 1) * HEAD_DIM],
                        xT[dt][:],
                        start=(dt == 0), stop=(dt == ndt - 1))
            for g in range(hkv):
                rope_drain(kTr[g][:, s0:s0 + SC], pks[g][:], cos_c, sin_c)

            # v projection (transposed), then PE-transpose to [s, d] bf16
            pvs = [ps_p.tile([P, SC], F32, tag="p", name=f"pv{c}_{i}")
                   for i in range(hkv)]
            for dt in range(ndt):
                wv = wkvp.tile([P, dkv], BF16, tag="wkv")
                nc.sync.dma_start(wv[:], wvT[dt * P:(dt + 1) * P, :])
                for g in range(hkv):
                    nc.tensor.matmul(
                        pvs[g][:],
                        wv[:, g * HEAD_DIM:(g + 1) * HEAD_DIM],
                        xT[dt][:],
                        start=(dt == 0), stop=(dt == ndt - 1))
            for g in range(hkv):
                vt_sb = wsb.tile([P, SC], BF16, tag="wsb")
                nc.any.tensor_copy(vt_sb[:], pvs[g][:])
                for st in range(SC // P):
                    pt = ps_p.tile([P, P], BF16, tag="t", bufs=1)
                    nc.tensor.transpose(pt[:], vt_sb[:, st * P:(st + 1) * P],
                                        iden_sb[:])
                    nc.any.tensor_copy(v_nat[g][c * (SC // P) + st][:], pt[:])

            # --- Phase 2: attention for this q-chunk
            nkt = (c + 1) * (SC // P)
            for h in range(hq):
                g = h // nrep
                acc = ws.tile([P, SC], F32, tag="ws")
                po = ps_a.tile([P, SC], F32, tag="o", bufs=3)
                for kt in range(nkt):
                    rr = kt * P - s0
                    jlo = max(0, rr)
                    pscr = ps_a.tile([P, SC], F32, tag="s")
                    nc.tensor.matmul(
                        pscr[:, jlo:SC],
                        kTr[g][:, kt * P:(kt + 1) * P],
                        qTr[h][:, s0 + jlo:s0 + SC],
                        start=True, stop=True)
                    et = wsb.tile([P, SC], BF16, tag="wsb")
                    nc.scalar.activation(
                        et[:, jlo:SC], pscr[:, jlo:SC],
                        mybir.ActivationFunctionType.Exp, scale=scale)
                    if rr >= 0:
                        nc.vector.tensor_mul(et[:, jlo:jlo + P],
                                             et[:, jlo:jlo + P], tri_sb[:])
                    if debug and c == 0 and h == 0 and kt == 0:
                        nc.sync.dma_start(dbg["det0"][:], et[:])
                    if kt == 0:
                        nc.vector.tensor_copy(_r(acc[:]), et[:])
                    else:
                        nc.vector.tensor_add(_r(acc[:, jlo:SC]),
                                             acc[:, jlo:SC], et[:, jlo:SC])
                    nc.tensor.matmul(
                        po[:, jlo:SC],
                        v_nat[g][kt][:],
                        et[:, jlo:SC],
                        start=(kt == 0), stop=(kt == nkt - 1))
                # denominator -> DRAM (normalization happens per-chunk below)
                pd = ps_a.tile([P, SC], F32, tag="s")
                nc.tensor.matmul(pd[0:1, :], ones_sb[:], _r(acc[:]),
                                 start=True, stop=True)
                dps = ws.tile([P, SC], F32, tag="ws")
                nc.scalar.copy(dps[0:1, :], pd[0:1, :])
                nc.sync.dma_start(dn_dram[h:h + 1, s0:s0 + SC], dps[0:1, :])
                # drain PV unnormalized
                nc.any.tensor_copy(ao[h][c][:], po[:])

            # batched normalization for this chunk (off critical path)
            dn_c = ws.tile([P, SC], F32, tag="ws")
            nc.sync.dma_start(dn_c[0:hq, :], dn_dram[:, s0:s0 + SC])
            rc_c = ws.tile([P, SC], F32, tag="ws")
            nc.vector.reciprocal(rc_c[0:hq, :], dn_c[0:hq, :])
            nc.sync.dma_start(dnr_dram[:, s0:s0 + SC], rc_c[0:hq, :])
            for h in range(hq):
                rrow = ws.tile([P, SC], F32, tag="ws")
                nc.sync.dma_start(rrow[0:1, :], dnr_dram[h:h + 1, s0:s0 + SC])
                rb = ws.tile([P, SC], F32, tag="ws")
                nc.gpsimd.partition_broadcast(rb[:], rrow[0:1, :])
                nc.vector.tensor_mul(ao[h][c][:], ao[h][c][:], rb[:])
            if debug and c == 0:
                nc.sync.dma_start(dbg["dao0"][:], ao[0][0][:])

        if debug:
            nc.sync.dma_start(dbg["dq0"][:], qTr[0][:])
            nc.sync.dma_start(dbg["dk0"][:], kTr[0][:])
            nc.sync.dma_start(dbg["dv0"][:], v_nat[0][0][:])
            nc.sync.dma_start(dbg["ddn"][:], dn_dram[:])

        # --- Phase 3: output projection from SBUF-resident att tiles
        for m in range(dim // P):
            wo = wop.tile([P, hq, P], BF16, tag="wo")
            nc.sync.dma_start(wo[:], wot[m].rearrange("(o p) f -> p o f", p=P))
            for cc in range(nch):
                py = ps_a.tile([P, SC], F32, tag="s")
                for o in range(hq):
                    nc.tensor.matmul(
                        py[:], wo[:, o, :], ao[o][cc][:],
                        start=(o == 0), stop=(o == hq - 1))
                yo = ws.tile([P, SC], F32, tag="ws")
                nc.any.tensor_copy(yo[:], py[:])
                nc.sync.dma_start(
                    outT[m * P:(m + 1) * P, cc * SC:(cc + 1) * SC], yo[:])

    nc.compile()
    return nc


def make_core_inputs(data, Wq, Wk, Wv, Wo, cos, sin):
    """Build in_maps for the 8 cores. Core id = 4*b + g."""
    bf = ml_dtypes.bfloat16

    def cbf(a):
        return np.ascontiguousarray(np.asarray(a).astype(bf))

    c = np.ascontiguousarray
    dq = HQ * HEAD_DIM
    dkv = HKV * HEAD_DIM
    dim = Wq.shape[1]
    tri_m = np.triu(np.ones((P, P), dtype=bf))
    iden = np.eye(P, dtype=bf)
    ones_col = np.ones((P, 1), dtype=np.float32)
    cosT = c(cos.T.astype(np.float32))
    sinT = c(sin.T.astype(np.float32))
    in_maps = []
    for core in range(N_CORES):
        b, g = divmod(core, 4)
        qs = slice(g * dq, (g + 1) * dq)
        ks = slice(g * dkv, (g + 1) * dkv)
        woT = Wo[:, qs].T                        # [dq, dim]
        wot = cbf(woT.reshape(dq, dim // P, P).transpose(1, 0, 2))
        in_maps.append({
            "x": cbf(data[b]),
            "wqT": cbf(Wq[qs, :].T),
            "wkT": cbf(Wk[ks, :].T),
            "wvT": cbf(Wv[ks, :].T),
            "wot": wot,
            "cosT": cosT,
            "sinT": sinT,
            "tri": tri_m,
            "iden": iden,
            "ones_col": ones_col,
        })
    return in_maps


_COMPILED = {}


def _get_program():
    key = (SEQ, DIM, HQ, HKV)
    if key not in _COMPILED:
        _COMPILED[key] = build_program()
    return _COMPILED[key]


def run(inputs, trace=False, tmpdir=None, trace_cores=None):
    nc = _get_program()
    in_maps = make_core_inputs(
        inputs["data"], inputs["Wq"], inputs["Wk"], inputs["Wv"],
        inputs["Wo"], inputs["cos"], inputs["sin"])
    kw = {}
    if trace:
        kw = dict(trace=True, tmpdir=tmpdir, trace_cores=trace_cores)
    res = run_bass_kernel_spmd(nc, in_maps, list(range(N_CORES)), **kw)
    B = inputs["data"].shape[0]
    out = np.zeros((B, SEQ, DIM), dtype=np.float32)
    for core in range(N_CORES):
        b = core // 4
        out[b] += res.results[core]["outT"].T
    return out, res


def kernel(data, Wq, Wk, Wv, Wo, cos, sin, mask):
    assert np.asarray(mask).size == 1, "only causal (numel==1) mask supported"
    inputs = {
        "data": np.asarray(data, dtype=np.float32),
        "Wq": np.asarray(Wq, dtype=np.float32),
        "Wk": np.asarray(Wk, dtype=np.float32),
        "Wv": np.asarray(Wv, dtype=np.float32),
        "Wo": np.asarray(Wo, dtype=np.float32),
        "cos": np.asarray(cos, dtype=np.float32),
        "sin": np.asarray(sin, dtype=np.float32),
    }
    out, _ = run(inputs)
    return out


# revision 10
# speedup vs baseline: 1.6182x; 1.3352x over previous
"""Trainium2 Bass kernel for nn_Attention_944892805701.

Dense transformer attention layer: QKV projection + RoPE + causal GQA SDPA +
output projection. B=2, S=2048, DIM=4096, 32 Q heads / 8 KV heads, hd=128.

Sharding (8 cores): 2 (batch) x 4 (head groups). Core (b, g) computes global
Q heads [8g, 8g+8) / KV heads [2g, 2g+2) of batch b and the corresponding
partial output projection y_partial = att_heads @ Wo[:, o_slice]^T. The host
sums the 4 head-group partials per batch (the tensor-parallel "allreduce",
done on host since full outputs are gathered there anyway).

Per-core device program: bf16 matmul operands (full PE rate + FWL weight
loads; fp32r measured ~2 cyc/row on HW so bf16 is 2x faster), fp32 PSUM
accumulation everywhere, fp32 softmax statistics.

  Phase 1 (per 512-wide s-chunk): xT tiles [din, s] via bf16 DMA transpose
    straight from DRAM (host pre-casts x to bf16), project qT/kT in
    [head_dim, s] layout (RoPE fused into the fp32 PSUM drain, bf16 out)
    and vT -> PE-transposed into natural [s, d] bf16 tiles.
  Phase 2 (per q-chunk, per head): scoresT = kT_tile x qT_chunk in
    [k-part, q-free] layout, exp on ScalarE (1/sqrt(hd) folded into the
    activation scale), causality via restricted column ranges plus one
    triangular mask multiply per diagonal block, denominator = fp32 DVE
    accumulation + fp32r ones-matmul partition reduce, PV accumulated in
    PSUM and drained UNNORMALIZED (bf16) to persistent SBUF tiles.
    Denominators go to DRAM ([1,512] rows); after each chunk one batched
    [8,512] reciprocal + partition_broadcast normalizes the chunk's ao
    tiles in place (off the per-head critical path).
  Phase 3: outT[m,:] = sum_o WoT[o-tile, m-tile].T @ att[o-tile, :] from
    SBUF-resident normalized bf16 attention tiles.

Output per core: outT [4096, 2048] f32 = y_partial^T; host transposes+sums.
"""

import math
from contextlib import ExitStack

import numpy as np
import ml_dtypes

import concourse.bass as bass  # noqa: F401
import concourse.tile as tile
from concourse import bacc, mybir
from concourse.bass_utils import run_bass_kernel_spmd

F32 = mybir.dt.float32
F32R = mybir.dt.float32r
BF16 = mybir.dt.bfloat16

N_CORES = 8
DIM = 4096
N_HEADS = 32
N_KV_HEADS = 8
HEAD_DIM = 128
SEQ = 2048

HQ = N_HEADS // 4      # 8 local q heads
HKV = N_KV_HEADS // 4  # 2 local kv heads

SC = 512
P = 128


def _r(ap):
    return ap.bitcast(F32R)


def build_program(seq=SEQ, dim=DIM, hq=HQ, hkv=HKV, debug=False):
    nrep = hq // hkv
    nch = seq // SC
    ndt = dim // P
    nkt_total = seq // P
    dq = hq * HEAD_DIM
    dkv = hkv * HEAD_DIM
    scale = 1.0 / math.sqrt(HEAD_DIM)

    nc = bacc.Bacc("TRN2", target_bir_lowering=False, debug=False,
                   num_devices=N_CORES)

    x = nc.dram_tensor("x", [seq, dim], BF16, kind="ExternalInput").ap()
    wqT = nc.dram_tensor("wqT", [dim, dq], BF16, kind="ExternalInput").ap()
    wkT = nc.dram_tensor("wkT", [dim, dkv], BF16, kind="ExternalInput").ap()
    wvT = nc.dram_tensor("wvT", [dim, dkv], BF16, kind="ExternalInput").ap()
    wot = nc.dram_tensor("wot", [dim // P, dq, P], BF16,
                         kind="ExternalInput").ap()
    cosT = nc.dram_tensor("cosT", [HEAD_DIM, seq], F32,
                          kind="ExternalInput").ap()
    sinT = nc.dram_tensor("sinT", [HEAD_DIM, seq], F32,
                          kind="ExternalInput").ap()
    tri = nc.dram_tensor("tri", [P, P], BF16, kind="ExternalInput").ap()
    iden = nc.dram_tensor("iden", [P, P], BF16, kind="ExternalInput").ap()
    ones_col = nc.dram_tensor("ones_col", [P, 1], F32R,
                              kind="ExternalInput").ap()
    outT = nc.dram_tensor("outT", [dim, seq], F32, kind="ExternalOutput").ap()
    dbg = {}
    if debug:
        for nm in ("dq0", "dk0"):
            dbg[nm] = nc.dram_tensor(nm, [P, seq], BF16,
                                     kind="ExternalOutput").ap()
        dbg["dv0"] = nc.dram_tensor("dv0", [P, HEAD_DIM], BF16,
                                    kind="ExternalOutput").ap()
        dbg["dao0"] = nc.dram_tensor("dao0", [P, SC], BF16,
                                     kind="ExternalOutput").ap()
        dbg["ddn"] = nc.dram_tensor("ddn", [hq, seq], F32,
                                    kind="ExternalOutput").ap()
        dbg["det0"] = nc.dram_tensor("det0", [P, SC], BF16,
                                     kind="ExternalOutput").ap()

    with ExitStack() as ctx:
        tc = ctx.enter_context(tile.TileContext(nc))
        ws = ctx.enter_context(tc.tile_pool(name="ws", bufs=16))    # f32 512
        wsb = ctx.enter_context(tc.tile_pool(name="wsb", bufs=76))  # bf16 512
        big = ctx.enter_context(tc.tile_pool(name="big", bufs=hq + hkv))
        vp = ctx.enter_context(tc.tile_pool(name="vp", bufs=hkv * nkt_total))
        wqp = ctx.enter_context(tc.tile_pool(name="wqp", bufs=4))
        wkvp = ctx.enter_context(tc.tile_pool(name="wkvp", bufs=6))
        wop = ctx.enter_context(tc.tile_pool(name="wop", bufs=3))
        cns = ctx.enter_context(tc.tile_pool(name="cns", bufs=1))
        ps_a = ctx.enter_context(tc.tile_pool(name="ps_a", bufs=2,
                                              space="PSUM"))
        ps_p = ctx.enter_context(tc.tile_pool(name="ps_p", bufs=2,
                                              space="PSUM"))
        dram = ctx.enter_context(tc.tile_pool(name="dram", bufs=1,
                                              space="DRAM"))

        dn_dram = dram.tile([hq, seq], F32, tag="dn")
        dnr_dram = dram.tile([hq, seq], F32, tag="dnr")

        tri_sb = cns.tile([P, P], BF16, tag="tri")
        nc.sync.dma_start(tri_sb[:], tri[:])
        iden_sb = cns.tile([P, P], BF16, tag="iden")
        nc.sync.dma_start(iden_sb[:], iden[:])
        ones_sb = cns.tile([P, 1], F32R, tag="ones")
        nc.sync.dma_start(ones_sb[:], ones_col[:])

        qTr = [big.tile([P, seq], BF16, tag="big", name=f"qTr{i}")
               for i in range(hq)]
        kTr = [big.tile([P, seq], BF16, tag="big", name=f"kTr{i}")
               for i in range(hkv)]
        v_nat = [[vp.tile([P, HEAD_DIM], BF16, tag="v", name=f"v{g}_{t}")
                  for t in range(nkt_total)] for g in range(hkv)]
        # unnormalized attention output tiles, persistent through phase 3
        ao = [[wsb.tile([P, SC], BF16, tag="wsb", name=f"ao{h}_{cc}")
               for cc in range(nch)] for h in range(hq)]

        def rope_drain(dst, psum, cos_c, sin_c):
            """dst(bf16) = psum*cos + rotate_half(psum)*sin."""
            h = HEAD_DIM // 2
            tmp = ws.tile([P, SC], F32, tag="ws")
            nc.vector.tensor_mul(dst, psum, cos_c[:])
            nc.vector.tensor_mul(tmp[0:h, :], psum[h:P, :], sin_c[0:h, :])
            nc.vector.tensor_mul(tmp[h:P, :], psum[0:h, :], sin_c[h:P, :])
            nc.vector.tensor_sub(dst[0:h, :], dst[0:h, :], tmp[0:h, :])
            nc.vector.tensor_add(dst[h:P, :], dst[h:P, :], tmp[h:P, :])

        for c in range(nch):
            s0 = c * SC

            cos_c = ws.tile([P, SC], F32, tag="ws")
            nc.sync.dma_start(cos_c[:], cosT[:, s0:s0 + SC])
            sin_c = ws.tile([P, SC], F32, tag="ws")
            nc.sync.dma_start(sin_c[:], sinT[:, s0:s0 + SC])

            # xT via bf16 DMA transpose straight from DRAM
            xT = [wsb.tile([P, SC], BF16, tag="wsb", name=f"xT{c}_{i}")
                  for i in range(ndt)]
            for dt in range(ndt):
                nc.sync.dma_start_transpose(
                    xT[dt][:], x[s0:s0 + SC, dt * P:(dt + 1) * P])

            # q projection: head pairs share one W block load
            for hb in range(0, hq, 2):
                pqs = [ps_p.tile([P, SC], F32, tag="p",
                                 name=f"pq{c}_{hb}_{i}") for i in range(2)]
                for dt4 in range(ndt // 4):
                    wq = wqp.tile([P, 4, 2 * HEAD_DIM], BF16, tag="wq")
                    nc.sync.dma_start(
                        wq[:], wqT[dt4 * 4 * P:(dt4 + 1) * 4 * P,
                                   hb * HEAD_DIM:(hb + 2) * HEAD_DIM
                                   ].rearrange("(d p) f -> p d f", p=P))
                    for j in range(4):
                        dt = dt4 * 4 + j
                        for i in range(2):
                            nc.tensor.matmul(
                                pqs[i][:],
                                wq[:, j, i * HEAD_DIM:(i + 1) * HEAD_DIM],
                                xT[dt][:],
                                start=(dt == 0), stop=(dt == ndt - 1))
                for i in range(2):
                    rope_drain(qTr[hb + i][:, s0:s0 + SC], pqs[i][:],
                               cos_c, sin_c)

            # k projection
            pks = [ps_p.tile([P, SC], F32, tag="p", name=f"pk{c}_{i}")
                   for i in range(hkv)]
            for dt4 in range(ndt // 4):
                wk = wkvp.tile([P, 4, dkv], BF16, tag="wkv")
                nc.sync.dma_start(
                    wk[:], wkT[dt4 * 4 * P:(dt4 + 1) * 4 * P, :
                               ].rearrange("(d p) f -> p d f", p=P))
                for j in range(4):
                    dt = dt4 * 4 + j
                    for g in range(hkv):
                        nc.tensor.matmul(
                            pks[g][:],
                            wk[:, j, g * HEAD_DIM:(g + 1) * HEAD_DIM],
                            xT[dt][:],
                            start=(dt == 0), stop=(dt == ndt - 1))
            for g in range(hkv):
                rope_drain(kTr[g][:, s0:s0 + SC], pks[g][:], cos_c, sin_c)

            # v projection (transposed), then PE-transpose to [s, d] bf16
            pvs = [ps_p.tile([P, SC], F32, tag="p", name=f"pv{c}_{i}")
                   for i in range(hkv)]
            for dt4 in range(ndt // 4):
                wv = wkvp.tile([P, 4, dkv], BF16, tag="wkv")
                nc.sync.dma_start(
                    wv[:], wvT[dt4 * 4 * P:(dt4 + 1) * 4 * P, :
                               ].rearrange("(d p) f -> p d f", p=P))
                for j in range(4):
                    dt = dt4 * 4 + j
                    for g in range(hkv):
                        nc.tensor.matmul(
                            pvs[g][:],
                            wv[:, j, g * HEAD_DIM:(g + 1) * HEAD_DIM],
                            xT[dt][:],
                            start=(dt == 0), stop=(dt == ndt - 1))
            for g in range(hkv):
                vt_sb = wsb.tile([P, SC], BF16, tag="wsb")
                nc.any.tensor_copy(vt_sb[:], pvs[g][:])
                for st in range(SC // P):
                    pt = ps_p.tile([P, P], BF16, tag="t", bufs=1)
                    nc.tensor.transpose(pt[:], vt_sb[:, st * P:(st + 1) * P],
                                        iden_sb[:])
                    nc.any.tensor_copy(v_nat[g][c * (SC // P) + st][:], pt[:])

            # --- Phase 2: attention for this q-chunk
            nkt = (c + 1) * (SC // P)
            for h in range(hq):
                g = h // nrep
                acc = ws.tile([P, SC], F32, tag="ws")
                po = ps_a.tile([P, SC], F32, tag="o", bufs=3)
                for kt in range(nkt):
                    rr = kt * P - s0
                    jlo = max(0, rr)
                    pscr = ps_a.tile([P, SC], F32, tag="s")
                    nc.tensor.matmul(
                        pscr[:, jlo:SC],
                        kTr[g][:, kt * P:(kt + 1) * P],
                        qTr[h][:, s0 + jlo:s0 + SC],
                        start=True, stop=True)
                    et = wsb.tile([P, SC], BF16, tag="wsb")
                    nc.scalar.activation(
                        et[:, jlo:SC], pscr[:, jlo:SC],
                        mybir.ActivationFunctionType.Exp, scale=scale)
                    if rr >= 0:
                        nc.vector.tensor_mul(et[:, jlo:jlo + P],
                                             et[:, jlo:jlo + P], tri_sb[:])
                    if debug and c == 0 and h == 0 and kt == 0:
                        nc.sync.dma_start(dbg["det0"][:], et[:])
                    if kt == 0:
                        nc.vector.tensor_copy(_r(acc[:]), et[:])
                    else:
                        nc.vector.tensor_add(_r(acc[:, jlo:SC]),
                                             acc[:, jlo:SC], et[:, jlo:SC])
                    nc.tensor.matmul(
                        po[:, jlo:SC],
                        v_nat[g][kt][:],
                        et[:, jlo:SC],
                        start=(kt == 0), stop=(kt == nkt - 1))
                # denominator -> DRAM (normalization happens per-chunk below)
                pd = ps_p.tile([P, SC], F32, tag="t", bufs=1)
                nc.tensor.matmul(pd[0:1, :], ones_sb[:], _r(acc[:]),
                                 start=True, stop=True)
                dps = ws.tile([P, SC], F32, tag="ws")
                nc.scalar.copy(dps[0:1, :], pd[0:1, :])
                nc.sync.dma_start(dn_dram[h:h + 1, s0:s0 + SC], dps[0:1, :])
                # drain PV unnormalized
                nc.any.tensor_copy(ao[h][c][:], po[:])

            # batched normalization for this chunk (off critical path)
            dn_c = ws.tile([P, SC], F32, tag="ws")
            nc.sync.dma_start(dn_c[0:hq, :], dn_dram[:, s0:s0 + SC])
            rc_c = ws.tile([P, SC], F32, tag="ws")
            nc.vector.reciprocal(rc_c[0:hq, :], dn_c[0:hq, :])
            nc.sync.dma_start(dnr_dram[:, s0:s0 + SC], rc_c[0:hq, :])
            for h in range(hq):
                rrow = ws.tile([P, SC], F32, tag="ws")
                nc.sync.dma_start(rrow[0:1, :], dnr_dram[h:h + 1, s0:s0 + SC])
                rb = ws.tile([P, SC], F32, tag="ws")
                nc.gpsimd.partition_broadcast(rb[:], rrow[0:1, :])
                nc.vector.tensor_mul(ao[h][c][:], ao[h][c][:], rb[:])
            if debug and c == 0:
                nc.sync.dma_start(dbg["dao0"][:], ao[0][0][:])

        if debug:
            nc.sync.dma_start(dbg["dq0"][:], qTr[0][:])
            nc.sync.dma_start(dbg["dk0"][:], kTr[0][:])
            nc.sync.dma_start(dbg["dv0"][:], v_nat[0][0][:])
            nc.sync.dma_start(dbg["ddn"][:], dn_dram[:])

        # --- Phase 3: output projection from SBUF-resident att tiles
        for m in range(dim // P):
            wo = wop.tile([P, hq, P], BF16, tag="wo")
            nc.scalar.dma_start(wo[:], wot[m].rearrange("(o p) f -> p o f", p=P))
            for cc in range(nch):
                py = ps_a.tile([P, SC], F32, tag="s")
                for o in range(hq):
                    nc.tensor.matmul(
                        py[:], wo[:, o, :], ao[o][cc][:],
                        start=(o == 0), stop=(o == hq - 1))
                yo = ws.tile([P, SC], F32, tag="ws")
                nc.vector.tensor_copy(yo[:], py[:])
                nc.scalar.dma_start(
                    outT[m * P:(m + 1) * P, cc * SC:(cc + 1) * SC], yo[:])

    nc.compile()
    return nc


def make_core_inputs(data, Wq, Wk, Wv, Wo, cos, sin):
    """Build in_maps for the 8 cores. Core id = 4*b + g."""
    bf = ml_dtypes.bfloat16

    def cbf(a):
        return np.ascontiguousarray(np.asarray(a).astype(bf))

    c = np.ascontiguousarray
    dq = HQ * HEAD_DIM
    dkv = HKV * HEAD_DIM
    dim = Wq.shape[1]
    tri_m = np.triu(np.ones((P, P), dtype=bf))
    iden = np.eye(P, dtype=bf)
    ones_col = np.ones((P, 1), dtype=np.float32)
    cosT = c(cos.T.astype(np.float32))
    sinT = c(sin.T.astype(np.float32))
    in_maps = []
    for core in range(N_CORES):
        b, g = divmod(core, 4)
        qs = slice(g * dq, (g + 1) * dq)
        ks = slice(g * dkv, (g + 1) * dkv)
        woT = Wo[:, qs].T                        # [dq, dim]
        wot = cbf(woT.reshape(dq, dim // P, P).transpose(1, 0, 2))
        in_maps.append({
            "x": cbf(data[b]),
            "wqT": cbf(Wq[qs, :].T),
            "wkT": cbf(Wk[ks, :].T),
            "wvT": cbf(Wv[ks, :].T),
            "wot": wot,
            "cosT": cosT,
            "sinT": sinT,
            "tri": tri_m,
            "iden": iden,
            "ones_col": ones_col,
        })
    return in_maps


_COMPILED = {}


def _get_program():
    key = (SEQ, DIM, HQ, HKV)
    if key not in _COMPILED:
        _COMPILED[key] = build_program()
    return _COMPILED[key]


def run(inputs, trace=False, tmpdir=None, trace_cores=None):
    nc = _get_program()
    in_maps = make_core_inputs(
        inputs["data"], inputs["Wq"], inputs["Wk"], inputs["Wv"],
        inputs["Wo"], inputs["cos"], inputs["sin"])
    kw = {}
    if trace:
        kw = dict(trace=True, tmpdir=tmpdir, trace_cores=trace_cores)
    res = run_bass_kernel_spmd(nc, in_maps, list(range(N_CORES)), **kw)
    B = inputs["data"].shape[0]
    out = np.zeros((B, SEQ, DIM), dtype=np.float32)
    for core in range(N_CORES):
        b = core // 4
        out[b] += res.results[core]["outT"].T
    return out, res


def kernel(data, Wq, Wk, Wv, Wo, cos, sin, mask):
    assert np.asarray(mask).size == 1, "only causal (numel==1) mask supported"
    inputs = {
        "data": np.asarray(data, dtype=np.float32),
        "Wq": np.asarray(Wq, dtype=np.float32),
        "Wk": np.asarray(Wk, dtype=np.float32),
        "Wv": np.asarray(Wv, dtype=np.float32),
        "Wo": np.asarray(Wo, dtype=np.float32),
        "cos": np.asarray(cos, dtype=np.float32),
        "sin": np.asarray(sin, dtype=np.float32),
    }
    out, _ = run(inputs)
    return out
